# revision 28
# baseline (speedup 1.0000x reference)
"""Trainium2 Bass kernel for HSEGNNFlexLayer (GNN message passing).

Strategy (8 NeuronCores, SPMD, minimal host<->device traffic):
  - Host assigns each node to a (core, window, slot) bin: 8 cores x 25
    windows x 256 slots.  Every edge is routed to the core that owns its
    dst node, so the segment-sum is fully local to each core.
  - x is shipped SHARDED (1/8 per core) and replicated on device with an
    HBM AllGather; x_i/x_j are then gathered on device per edge window
    with gpsimd dma_gather (transposed layout, features on partitions).
    int16 gather indices only span 32768 rows, so the padded x table has
    a zero row in both a low [0, 32K) and a high [N_pad-32K, N_pad)
    window; each edge gathers from both windows (the miss hits a zero
    row) and the two results are summed.
  - The scatter one-hot S is built on device from per-edge slot ids via
    iota + is_equal, removing the [E, 256] host-staged matrix.
  - Weights travel sharded through a second AllGather; edge/node attrs
    travel in bf16 and are widened on device.
  - Message layers: c = a @ Wflat with edges on PSUM partitions,
    attr-weighted k-sum via scalar_tensor_tensor chains, Silu on
    ScalarE; scatter-add via one-hot matmul into a per-window PSUM bank.
"""

import os
import tempfile

import numpy as np
import ml_dtypes

import jax

_cc_dir = os.path.join(tempfile.gettempdir(), "jax_cc_cache")
jax.config.update("jax_compilation_cache_dir", _cc_dir)
jax.config.update("jax_persistent_cache_min_entry_size_bytes", -1)
jax.config.update("jax_persistent_cache_min_compile_time_secs", 0)

import concourse.bass as bass
import concourse.mybir as mybir
import concourse.tile as tile
from concourse import bacc
from concourse import bass_utils
from concourse.masks import make_identity

# Problem constants (hardcoded per contest contract)
N, E, D, A, AM = 50000, 500000, 128, 8, 3
MIN_DIM = 2 * D + AM  # 259
UIN_DIM = D + D + AM  # 259
NCORES = 8
P = 128
KO = A * D  # 1024 flattened (k, o) output columns per TP layer
SLOTS = 256  # node slots per window
NWIN = 25
NODE_SLOTS = NWIN * SLOTS  # 6400 per core
NNT = NODE_SLOTS // P  # 50 node tiles

# Padded x table: row 0 = zero row (low window), row N+1 = zero row
# (high window), padded to a multiple of NCORES for the AllGather shard.
NROWS_PAD = 50304
XSHARD = NROWS_PAD // NCORES  # 6288
LO_SIZE = 32768
HI_OFF = NROWS_PAD - LO_SIZE  # 17536
ZHI = N + 1  # index of the high-window zero row (50001)

# Weight blob rows: w1(259) + w2(128) + w3(259) + w4(128) = 774, pad 776
WROWS = 776
WSHARD = WROWS // NCORES  # 97
W1_OFF, W2_OFF, W3_OFF, W4_OFF = 0, 259, 387, 646

BF16 = mybir.dt.bfloat16
F32 = mybir.dt.float32
I16 = mybir.dt.int16
NPBF16 = ml_dtypes.bfloat16

_cache = {}


def _blob_layout(T_B):
    """Byte-identical input blob layout shared by host packing and the
    device kernel.  All sections are 2-byte dtypes (bf16 / int16); offsets
    are in 2-byte elements, 256-element (512 B) aligned."""
    win_cap = T_B * P
    E_pad = NWIN * win_cap
    C = win_cap // 16
    CN = NODE_SLOTS // 16
    sections = [
        ("xshard", XSHARD, D),
        ("wshard", WSHARD, KO),
        ("gidx", NWIN * 16, 2 * C),
        ("slotsW", NWIN * P, T_B),
        ("battrW", NWIN * P, T_B * A),
        ("amfW", AM, E_pad),
        ("nidx", 16, 2 * CN),
        ("nanfT", AM, NODE_SLOTS),
        ("nattrW", P, NNT * A),
        ("bias", 4, D),
    ]
    layout = {}
    off = 0
    for name, r, c in sections:
        layout[name] = (off, r, c)
        off += -(r * c) // -256 * 256  # round up to 256 elements
    return layout, off


# --------------------------------------------------------------------------
# Host-side preparation
# --------------------------------------------------------------------------

def _assign_nodes(dst):
    """Snake round-robin of nodes (sorted by edge count desc) over the
    NCORES*NWIN bins: balances per-bin edge counts, gives every bin
    exactly ceil(N/nbins) <= SLOTS nodes, fully vectorized."""
    counts = np.bincount(dst, minlength=N)
    order = np.argsort(-counts, kind="stable")
    nbins = NCORES * NWIN
    k = np.arange(N)
    rnd, pos = k // nbins, k % nbins
    bins = np.where(rnd % 2 == 0, pos, nbins - 1 - pos).astype(np.int32)
    node2bin = np.empty(N, dtype=np.int32)
    node2slot = np.empty(N, dtype=np.int32)
    node2bin[order] = bins
    node2slot[order] = rnd.astype(np.int32)
    return node2bin, node2slot


def _wrap16(a):
    """[NWIN, cap] -> [NWIN, 16, cap//16] int16: index i of each window
    wrapped to (partition i%16, column i//16).  The device replicates the
    16-partition block to all 8 gpsimd groups."""
    nwin, cap = a.shape
    return np.ascontiguousarray(
        a.reshape(nwin, cap // 16, 16).transpose(0, 2, 1).astype(np.int16))


def _split_idx(id1):
    """Padded-table row ids -> (lo, hi) int16 gather indices.  Whichever
    window does not contain the row points at that window's zero row."""
    lo = np.where(id1 < LO_SIZE, id1, 0).astype(np.int16)
    hi = np.where(id1 >= LO_SIZE, id1 - HI_OFF, ZHI - HI_OFF).astype(np.int16)
    return lo, hi


def _prepare(x, edge_attr, node_attr, amf, anf, W1, b1, W2, b2, W3, b3, W4, b4,
             edge_index):
    x = np.asarray(x, dtype=np.float32)
    edge_attr = np.asarray(edge_attr, dtype=np.float32)
    node_attr = np.asarray(node_attr, dtype=np.float32)
    amf = np.asarray(amf, dtype=np.float32)
    anf = np.asarray(anf, dtype=np.float32)
    src = np.asarray(edge_index[0], dtype=np.int64).astype(np.int32)
    dst = np.asarray(edge_index[1], dtype=np.int64).astype(np.int32)

    node2bin, node2slot = _assign_nodes(dst)
    node_core = node2bin // NWIN
    node_gslot = (node2bin % NWIN) * SLOTS + node2slot

    e_bin = node2bin[dst]
    e_order = np.argsort(e_bin, kind="stable")
    e_bin_sorted = e_bin[e_order]
    bin_cnt = np.bincount(e_bin_sorted, minlength=NCORES * NWIN)
    T_B = int(np.ceil(bin_cnt.max() / P))
    win_cap = T_B * P
    E_pad = NWIN * win_cap

    bin_starts = np.zeros(NCORES * NWIN + 1, dtype=np.int64)
    np.cumsum(bin_cnt, out=bin_starts[1:])
    offs_in_bin = np.arange(len(e_order)) - bin_starts[e_bin_sorted]
    pos = (e_bin_sorted % NWIN) * win_cap + offs_in_bin
    core_of_edge = e_bin_sorted // NWIN
    gpos = core_of_edge * E_pad + pos

    src_s, dst_s = src[e_order], dst[e_order]

    # per-(core,slot) padded edge arrays; 0 / -1 sentinels for padding
    id1s = np.zeros(NCORES * E_pad, np.int32)
    id1d = np.zeros(NCORES * E_pad, np.int32)
    slots_a = np.full(NCORES * E_pad, -1.0, np.float32)
    battr_a = np.zeros((NCORES * E_pad, A), np.float32)
    amf_a = np.zeros((NCORES * E_pad, AM), np.float32)
    id1s[gpos] = src_s + 1
    id1d[gpos] = dst_s + 1
    slots_a[gpos] = node2slot[dst_s]
    battr_a[gpos] = edge_attr[e_order]
    amf_a[gpos] = amf[e_order]

    id1s = id1s.reshape(NCORES, NWIN, win_cap)
    id1d = id1d.reshape(NCORES, NWIN, win_cap)
    C = win_cap // 16

    # padded x table (shared across cores; shipped sharded)
    xstage = np.zeros((NROWS_PAD, D), NPBF16)
    xstage[1:N + 1] = x.astype(NPBF16)

    # weight blob
    wblob = np.zeros((WROWS, KO), NPBF16)
    wblob[W1_OFF:W1_OFF + MIN_DIM] = np.asarray(W1, np.float32).reshape(MIN_DIM, KO).astype(NPBF16)
    wblob[W2_OFF:W2_OFF + D] = np.asarray(W2, np.float32).reshape(D, KO).astype(NPBF16)
    wblob[W3_OFF:W3_OFF + UIN_DIM] = np.asarray(W3, np.float32).reshape(UIN_DIM, KO).astype(NPBF16)
    wblob[W4_OFF:W4_OFF + D] = np.asarray(W4, np.float32).reshape(D, KO).astype(NPBF16)

    bias4 = np.stack([np.asarray(b, np.float32) for b in (b1, b2, b3, b4)]
                     ).astype(NPBF16)  # [4, D]

    layout, blob_elems = _blob_layout(T_B)

    def pack(parts):
        blob = np.zeros(blob_elems, np.int16)
        for name, arr in parts.items():
            off, r, c = layout[name]
            blob[off:off + r * c] = arr.view(np.int16).ravel()
        return blob

    in_maps = []
    slot2node = np.full((NCORES, NODE_SLOTS), -1, dtype=np.int64)
    for c in range(NCORES):
        slo, shi = _split_idx(id1s[c])
        gidx = np.concatenate(
            [_wrap16(a) for a in (slo, shi)], axis=2)  # [NWIN,16,2C]
        gidx = np.ascontiguousarray(gidx.reshape(NWIN * 16, 2 * C))

        slotsW = slots_a.reshape(NCORES, NWIN, T_B, P)[c].transpose(0, 2, 1)
        slotsW = np.ascontiguousarray(slotsW.reshape(NWIN * P, T_B).astype(NPBF16))
        battrW = battr_a.reshape(NCORES, NWIN, T_B, P, A)[c].transpose(0, 2, 1, 3)
        battrW = np.ascontiguousarray(battrW.reshape(NWIN * P, T_B * A).astype(NPBF16))
        amfW = np.ascontiguousarray(
            amf_a.reshape(NCORES, E_pad, AM)[c].T.astype(NPBF16))

        # node side
        nodes_c = np.nonzero(node_core == c)[0]
        gs = node_gslot[nodes_c]
        slot2node[c, gs] = nodes_c
        nid1 = np.zeros(NODE_SLOTS, np.int32)
        nid1[gs] = nodes_c + 1
        nlo, nhi = _split_idx(nid1[None, :])
        nidx = np.ascontiguousarray(np.concatenate(
            [_wrap16(nlo)[0], _wrap16(nhi)[0]], axis=1))  # [16, 2*CN]
        nanfT = np.zeros((AM, NODE_SLOTS), NPBF16)
        nanfT[:, gs] = anf[nodes_c].T.astype(NPBF16)
        nattr_a = np.zeros((NODE_SLOTS, A), np.float32)
        nattr_a[gs] = node_attr[nodes_c]
        nattrW = np.ascontiguousarray(
            nattr_a.reshape(NNT, P, A).transpose(1, 0, 2).reshape(P, NNT * A).astype(NPBF16))

        in_maps.append({"blob": pack({
            "xshard": np.ascontiguousarray(xstage[c * XSHARD:(c + 1) * XSHARD]),
            "wshard": np.ascontiguousarray(wblob[c * WSHARD:(c + 1) * WSHARD]),
            "gidx": gidx,
            "slotsW": slotsW,
            "battrW": battrW,
            "amfW": amfW,
            "nidx": nidx,
            "nanfT": np.ascontiguousarray(nanfT),
            "nattrW": nattrW,
            "bias": bias4,
        })})
    return in_maps, slot2node, T_B


# --------------------------------------------------------------------------
# Device kernel builder
# --------------------------------------------------------------------------

def _build(T_B):
    win_cap = T_B * P
    E_pad = NWIN * win_cap
    C = win_cap // 16
    CN = NODE_SLOTS // 16

    nc = bacc.Bacc("TRN2", target_bir_lowering=False, debug=False,
                   num_devices=NCORES)

    layout, blob_elems = _blob_layout(T_B)
    d_blob = nc.dram_tensor("blob", [blob_elems], I16, kind="ExternalInput")

    def sect(name, dtype):
        off, r, c = layout[name]
        ap = bass.AP(d_blob.ap().tensor, off, [[c, r], [1, c]])
        return ap if dtype == I16 else ap.bitcast(dtype)

    d_xshard = sect("xshard", BF16)
    d_wshard = sect("wshard", BF16)
    d_gidx = sect("gidx", I16)
    d_slotsW = sect("slotsW", BF16)
    d_battrW = sect("battrW", BF16)
    d_amfW = sect("amfW", BF16)
    d_nidx = sect("nidx", I16)
    d_nanfT = sect("nanfT", BF16)
    d_nattrW = sect("nattrW", BF16)
    d_bias = sect("bias", BF16)
    d_out = nc.dram_tensor("out", [NODE_SLOTS, D], BF16, kind="ExternalOutput")

    d_xsh_i = nc.dram_tensor("xsh_i", [XSHARD, D], BF16)
    d_xfull = nc.dram_tensor("xfull", [NROWS_PAD, D], BF16, addr_space="Shared")
    d_wsh_i = nc.dram_tensor("wsh_i", [WSHARD, KO], BF16)
    d_wfull = nc.dram_tensor("wfull", [WROWS, KO], BF16, addr_space="Shared")

    mult = mybir.AluOpType.mult
    add = mybir.AluOpType.add
    iseq = mybir.AluOpType.is_equal
    silu = mybir.ActivationFunctionType.Silu
    groups = [list(range(NCORES))]

    with tile.TileContext(nc) as tc:
        with (
            tc.tile_pool(name="const", bufs=1) as cpool,
            tc.tile_pool(name="ain", bufs=2) as apool,
            tc.tile_pool(name="gth", bufs=2) as gpool,
            tc.tile_pool(name="work", bufs=3) as wpool,
            tc.tile_pool(name="cps", bufs=2, space="PSUM") as cps,
            tc.tile_pool(name="trps", bufs=2, space="PSUM") as trps,
            tc.tile_pool(name="aggps", bufs=1, space="PSUM") as aggps,
        ):
            # ---- replicate x and weights across cores ----
            nc.sync.dma_start(d_xsh_i.ap(), d_xshard)
            nc.gpsimd.collective_compute(
                "AllGather", mybir.AluOpType.bypass, groups,
                ins=[d_xsh_i.ap()], outs=[d_xfull.ap()])
            nc.sync.dma_start(d_wsh_i.ap(), d_wshard)
            nc.gpsimd.collective_compute(
                "AllGather", mybir.AluOpType.bypass, groups,
                ins=[d_wsh_i.ap()], outs=[d_wfull.ap()])

            x_lo = d_xfull.ap()[0:LO_SIZE, :]
            x_hi = d_xfull.ap()[HI_OFF:NROWS_PAD, :]

            # ---- constants resident in SBUF ----
            ident = cpool.tile([P, P], BF16, tag="ident", name="ident")
            make_identity(nc, ident[:])

            def wtile(rows, off, tag):
                t = cpool.tile([rows, KO], BF16, tag=tag, name=tag)
                nc.sync.dma_start(t[:], d_wfull.ap()[off:off + rows, :])
                return t

            w1c = [wtile(P, W1_OFF, "w1c0"), wtile(P, W1_OFF + P, "w1c1"),
                   wtile(AM, W1_OFF + 2 * P, "w1c2")]
            w2c = [wtile(P, W2_OFF, "w2c")]
            w3c = [wtile(P, W3_OFF, "w3c0"), wtile(P, W3_OFF + P, "w3c1"),
                   wtile(AM, W3_OFF + 2 * P, "w3c2")]
            w4c = [wtile(P, W4_OFF, "w4c")]

            # biases: [4, D] bf16 rows -> [P, D] f32 via K=1 ones matmul
            ones1 = cpool.tile([1, P], BF16, tag="ones1", name="ones1")
            nc.vector.memset(ones1[:], 1.0)
            btile = [cpool.tile([P, D], F32, tag=f"b{i}r", name=f"b{i}r")
                     for i in range(4)]
            for i in range(4):
                brow = cpool.tile([1, D], BF16, tag=f"brow{i}", name=f"brow{i}")
                nc.sync.dma_start(brow[:], d_bias[i:i + 1, :])
                bps = aggps.tile([P, D], F32, tag="bps", name="bps")
                nc.tensor.matmul(bps[:], lhsT=ones1[:], rhs=brow[:],
                                 start=True, stop=True)
                nc.vector.tensor_copy(btile[i][:], bps[:])

            cols_i = cpool.tile([P, SLOTS], mybir.dt.int32, tag="colsi", name="colsi")
            nc.gpsimd.iota(cols_i[:], pattern=[[1, SLOTS]], base=0,
                           channel_multiplier=0)
            cols_f = cpool.tile([P, SLOTS], F32, tag="colsf", name="colsf")
            nc.vector.tensor_copy(cols_f[:], cols_i[:])

            # per-partition row index columns (for the transposed one-hot ST)
            prow_i = cpool.tile([P, 2], mybir.dt.int32, tag="prowi", name="prowi")
            nc.gpsimd.iota(prow_i[:], pattern=[[128, 2]], base=0,
                           channel_multiplier=1)
            prow_f = cpool.tile([P, 2], F32, tag="prowf", name="prowf")
            nc.vector.tensor_copy(prow_f[:], prow_i[:])

            aggT = cpool.tile([P, NODE_SLOTS], BF16, tag="aggT", name="aggT")

            # ---- helper: one TP layer tile ----
            def tp_layer(chunks, wchunks, bt, bias_rep, out_tile, do_silu):
                cpsum = cps.tile([P, KO], F32, tag="c", name="c")
                nch = len(chunks)
                for ci in range(nch):
                    for h in range(2):
                        nc.tensor.matmul(
                            cpsum[:, h * 512:(h + 1) * 512],
                            lhsT=chunks[ci],
                            rhs=wchunks[ci][:, h * 512:(h + 1) * 512],
                            start=(ci == 0),
                            stop=(ci == nch - 1),
                        )
                acc = wpool.tile([P, D], F32, tag="acc", name="acc")
                nc.vector.scalar_tensor_tensor(
                    acc[:], cpsum[:, 0:D], bt[:, 0:1], bias_rep[:], mult, add)
                for k in range(1, A):
                    nc.vector.scalar_tensor_tensor(
                        acc[:], cpsum[:, k * D:(k + 1) * D], bt[:, k:k + 1],
                        acc[:], mult, add)
                if do_silu:
                    nc.scalar.activation(out_tile[:], acc[:], silu)
                else:
                    nc.vector.tensor_copy(out_tile[:], acc[:])

            def transpose_to(src_bf16):
                tps = trps.tile([P, P], BF16, tag="tr", name="tr")
                nc.tensor.transpose(tps[:], src_bf16[:], ident[:])
                dst = wpool.tile([P, P], BF16, tag="mT", name="mT")
                nc.vector.tensor_copy(dst[:], tps[:])
                return dst

            def rep16(dst_tile, dram_ap):
                """Load a [16, X] int16 DRAM block into all 8 16-partition
                groups of dst_tile (dma_gather reads indices per-group)."""
                for k in range(8):
                    nc.sync.dma_start(dst_tile[16 * k:16 * (k + 1), :], dram_ap)

            def gather_pair(dst_tile, lo_cols, hi_cols, idxt, n_idx, tag):
                """dst = xfull[ids] via dual-window dma_gather + add."""
                tmp = gpool.tile(list(dst_tile.shape), BF16, tag=tag, name=tag)
                nc.gpsimd.dma_gather(
                    dst_tile[:].unsqueeze(1), x_lo, idxt[:, lo_cols[0]:lo_cols[1]],
                    num_idxs=n_idx, num_idxs_reg=n_idx, elem_size=D,
                    transpose=True, single_packet=False)
                nc.gpsimd.dma_gather(
                    tmp[:].unsqueeze(1), x_hi, idxt[:, hi_cols[0]:hi_cols[1]],
                    num_idxs=n_idx, num_idxs_reg=n_idx, elem_size=D,
                    transpose=True, single_packet=False)
                nc.vector.tensor_tensor(dst_tile[:], dst_tile[:], tmp[:], add)

            # ---- owned-node features (needed by both phases) ----
            nidxt = cpool.tile([P, 2 * CN], I16, tag="nidxt", name="nidxt")
            rep16(nidxt, d_nidx)
            nxT = cpool.tile([P, NODE_SLOTS], BF16, tag="nxT", name="nxT")
            gather_pair(nxT, (0, CN), (CN, 2 * CN), nidxt, NODE_SLOTS, "gnx")

            # ---- edge phase ----
            for w in range(NWIN):
                idxt = apool.tile([P, 2 * C], I16, tag="idxt", name="idxt")
                rep16(idxt, d_gidx[w * 16:(w + 1) * 16, :])
                slt_b = apool.tile([P, T_B], BF16, tag="sltb", name="sltb")
                nc.sync.dma_start(slt_b[:], d_slotsW[w * P:(w + 1) * P, :])
                slt = wpool.tile([P, T_B], F32, tag="sltf", name="sltf")
                nc.vector.tensor_copy(slt[:], slt_b[:])
                bat_b = apool.tile([P, T_B * A], BF16, tag="batb", name="batb")
                nc.sync.dma_start(bat_b[:], d_battrW[w * P:(w + 1) * P, :])
                bat = wpool.tile([P, T_B * A], F32, tag="batf", name="batf")
                nc.vector.tensor_copy(bat[:], bat_b[:])
                amt = apool.tile([AM, win_cap], BF16, tag="amt", name="amt")
                nc.sync.dma_start(
                    amt[:], d_amfW[:, w * win_cap:(w + 1) * win_cap])

                # Ya[s, ko] = sum_d xown[d, w*SLOTS+s] * W1a[d, ko]; the
                # per-edge x_i contribution is then ST-selected by slot.
                ya = []
                for h in range(2):
                    ya_ps = cps.tile([P, KO], F32, tag="c", name="c")
                    for q in range(2):
                        nc.tensor.matmul(
                            ya_ps[:, q * 512:(q + 1) * 512],
                            lhsT=nxT[:, w * SLOTS + h * P:w * SLOTS + (h + 1) * P],
                            rhs=w1c[0][:, q * 512:(q + 1) * 512],
                            start=True, stop=True)
                    ya_sb = apool.tile([P, KO], BF16, tag=f"ya{h}", name=f"ya{h}")
                    nc.vector.tensor_copy(ya_sb[:], ya_ps[:])
                    ya.append(ya_sb)

                xj = gpool.tile([P, win_cap], BF16, tag="xj", name="xj")
                gather_pair(xj, (0, C), (C, 2 * C), idxt, win_cap, "ghj")

                agg_ps = aggps.tile([P, SLOTS], F32, tag="agg", name="agg")
                for j in range(T_B):
                    # transposed one-hot: ST[s, e] = (slot[e] == prow[s])
                    tt_ps = trps.tile([P, P], BF16, tag="tr", name="tr")
                    nc.tensor.transpose(
                        tt_ps[:], slt_b[:, j:j + 1].to_broadcast([P, P]),
                        ident[:])
                    ttf = wpool.tile([P, P], F32, tag="ttf", name="ttf")
                    nc.vector.tensor_copy(ttf[:], tt_ps[:])
                    st_lo = wpool.tile([P, P], BF16, tag="stlo", name="stlo")
                    nc.vector.tensor_scalar(
                        st_lo[:], ttf[:], prow_f[:, 0:1], None, iseq)
                    st_hi = wpool.tile([P, P], BF16, tag="sthi", name="sthi")
                    nc.vector.tensor_scalar(
                        st_hi[:], ttf[:], prow_f[:, 1:2], None, iseq)

                    m1 = wpool.tile([P, D], BF16, tag="m1", name="m1")
                    tp_layer([st_lo, st_hi,
                              xj[:, j * P:(j + 1) * P],
                              amt[:, j * P:(j + 1) * P]],
                             [ya[0], ya[1], w1c[1], w1c[2]],
                             bat[:, j * A:(j + 1) * A], btile[0], m1, True)
                    m1T = transpose_to(m1)
                    m2 = wpool.tile([P, D], BF16, tag="m2", name="m2")
                    tp_layer([m1T], w2c, bat[:, j * A:(j + 1) * A], btile[1],
                             m2, True)

                    St = wpool.tile([P, SLOTS], BF16, tag="St", name="St")
                    nc.vector.tensor_scalar(
                        St[:], cols_f[:], slt[:, j:j + 1], None, iseq)
                    nc.tensor.matmul(
                        agg_ps[:], lhsT=m2[:], rhs=St[:],
                        start=(j == 0), stop=(j == T_B - 1))
                nc.vector.tensor_copy(
                    aggT[:, w * SLOTS:(w + 1) * SLOTS], agg_ps[:])

            # ---- node phase ----
            nanf = cpool.tile([AM, NODE_SLOTS], BF16, tag="nanf", name="nanf")
            nc.sync.dma_start(nanf[:], d_nanfT)
            nat_b = cpool.tile([P, NNT * A], BF16, tag="natb", name="natb")
            nc.sync.dma_start(nat_b[:], d_nattrW)
            nat = cpool.tile([P, NNT * A], F32, tag="natf", name="natf")
            nc.vector.tensor_copy(nat[:], nat_b[:])

            for t in range(NNT):
                u = wpool.tile([P, D], BF16, tag="m1", name="m1")
                tp_layer([nxT[:, t * P:(t + 1) * P],
                          aggT[:, t * P:(t + 1) * P],
                          nanf[:, t * P:(t + 1) * P]],
                         w3c, nat[:, t * A:(t + 1) * A], btile[2], u, True)
                uT = transpose_to(u)
                out_t = wpool.tile([P, D], BF16, tag="outt", name="outt")
                tp_layer([uT], w4c, nat[:, t * A:(t + 1) * A], btile[3],
                         out_t, False)
                nc.sync.dma_start(d_out.ap()[t * P:(t + 1) * P, :], out_t[:])

    nc.compile()
    return nc


# --------------------------------------------------------------------------
# Cached PJRT runner
#
# bass_utils.run_bass_kernel_spmd re-wraps the NEFF in a fresh jax.jit on
# every call, so the terminal re-loads the (multi-MB) NEFF each time.
# Building the jit executable once per compiled kernel keeps the NEFF
# loaded; repeated calls then only pay input transfer + execution.
# --------------------------------------------------------------------------

def _make_runner(nc):
    """jit-once PJRT runner.  Unlike run_bass_via_pjrt it (a) reuses one
    loaded executable across calls (no per-call NEFF reload over the
    link) and (b) skips the donated zero output buffers — this kernel
    writes every output element, so uninitialized result buffers are
    fine and the zero upload is dead weight."""
    from concourse import bass2jax
    bass2jax.install_neuronx_cc_hook()
    assert not getattr(nc, "dbg_callbacks", None)
    partition_name = nc.partition_id_tensor.name if nc.partition_id_tensor else None

    in_names, out_names, out_avals = [], [], []
    for alloc in nc.m.functions[0].allocations:
        if not isinstance(alloc, mybir.MemoryLocationSet):
            continue
        name = alloc.memorylocations[0].name
        if alloc.kind == "ExternalInput":
            if name != partition_name:
                in_names.append(name)
        elif alloc.kind == "ExternalOutput":
            out_names.append(name)
            shape = tuple(alloc.tensor_shape)
            dtype = mybir.dt.np(alloc.dtype)
            out_avals.append(jax.core.ShapedArray(shape, dtype))
    all_in_names = in_names + ([partition_name] if partition_name else [])

    def _body(*args):
        operands = list(args)
        if partition_name is not None:
            operands.append(bass2jax.partition_id_tensor())
        outs = bass2jax._bass_exec_p.bind(
            *operands, out_avals=tuple(out_avals), in_names=tuple(all_in_names),
            out_names=tuple(out_names), lowering_input_output_aliases=(),
            sim_require_finite=True, sim_require_nnan=True, nc=nc)
        return tuple(outs)

    devices = jax.devices()[:NCORES]
    mesh = bass2jax.Mesh(np.asarray(devices), ("core",))
    in_specs = (bass2jax.PartitionSpec("core"),) * len(in_names)
    out_specs = (bass2jax.PartitionSpec("core"),) * len(out_names)
    sharded = jax.jit(
        bass2jax.shard_map(_body, mesh=mesh, in_specs=in_specs,
                           out_specs=out_specs, check_rep=False),
        keep_unused=True)

    def run(concat_in):
        out_arrs = sharded(*concat_in)
        return [
            {name: np.asarray(out_arrs[i]).reshape(NCORES, *out_avals[i].shape)[c]
             for i, name in enumerate(out_names)}
            for c in range(NCORES)
        ]

    run.in_names = in_names
    return run


def _concat_inputs(run, in_maps):
    return [np.concatenate([np.asarray(in_maps[c][name])
                            for c in range(NCORES)], axis=0)
            for name in run.in_names]


# --------------------------------------------------------------------------
# Entry point
# --------------------------------------------------------------------------

def kernel(x, edge_attr, node_attr, additional_message_features,
           additional_node_features, W1, b1, W2, b2, W3, b3, W4, b4,
           edge_index, batch=None):
    in_maps, slot2node, T_B = _prepare(
        x, edge_attr, node_attr, additional_message_features,
        additional_node_features, W1, b1, W2, b2, W3, b3, W4, b4, edge_index)

    if T_B not in _cache:
        nc = _build(T_B)
        _cache[T_B] = (nc, _make_runner(nc))
    nc, run = _cache[T_B]

    concat_in = _concat_inputs(run, in_maps)
    results = run(concat_in)
    kernel.last = (run, concat_in, results)

    out = np.zeros((N, D), dtype=np.float32)
    for c in range(NCORES):
        oc = np.asarray(results[c]["out"], dtype=np.float32)
        mask = slot2node[c] >= 0
        out[slot2node[c][mask]] = oc[mask]
    return out


# revision 29
# speedup vs baseline: 1.0044x; 1.0044x over previous
"""Trainium2 Bass kernel for HSEGNNFlexLayer (GNN message passing).

Strategy (8 NeuronCores, SPMD, transfer-minimal):
  - Host assigns each node to a (core, window, slot) bin: 8 cores x 25
    windows x 256 slots.  Every edge is routed to the core that owns its
    dst node, so the segment-sum is fully local to each core.
  - All per-core inputs travel in ONE 2-byte-element blob (~3.8 MB/core);
    x and the weights travel sharded and are replicated on device via HBM
    AllGather collectives.
  - x_j is gathered per edge window on device with gpsimd dma_gather
    (transposed layout, features on partitions).  int16 gather indices
    only span 32768 rows, so the padded x table keeps a zero row in both
    a low and a high 32K window; each edge gathers from both windows
    (the miss hits a zero row) and the two results are summed.
  - x_i never needs a per-edge gather: each window's destinations are its
    <=256 owned slots, so Ya = xown^T @ W1a is computed once per window
    and injected per-edge through a transposed slot one-hot (ST) matmul
    chunk.  The scatter-add one-hot S is likewise built on device from
    slot ids via iota + is_equal.
  - TP layers: c = a @ Wflat with edges on PSUM partitions, attr-weighted
    k-sum via scalar_tensor_tensor chains, Silu on ScalarE; scatter-add
    via one-hot matmul accumulated in a per-window PSUM bank.
  - A jit-once PJRT runner keeps the loaded NEFF resident across calls
    and skips donated zero output buffers (the kernel writes every output
    element), so warm calls pay only input transfer + execution.
"""

import os
import tempfile

import numpy as np
import ml_dtypes

import jax

_cc_dir = os.path.join(tempfile.gettempdir(), "jax_cc_cache")
jax.config.update("jax_compilation_cache_dir", _cc_dir)
jax.config.update("jax_persistent_cache_min_entry_size_bytes", -1)
jax.config.update("jax_persistent_cache_min_compile_time_secs", 0)

import concourse.bass as bass
import concourse.mybir as mybir
import concourse.tile as tile
from concourse import bacc
from concourse.masks import make_identity

# Problem constants (hardcoded per contest contract)
N, E, D, A, AM = 50000, 500000, 128, 8, 3
MIN_DIM = 2 * D + AM  # 259
UIN_DIM = D + D + AM  # 259
NCORES = 8
P = 128
KO = A * D  # 1024 flattened (k, o) output columns per TP layer
SLOTS = 256  # node slots per window
NWIN = 25
NODE_SLOTS = NWIN * SLOTS  # 6400 per core
NNT = NODE_SLOTS // P  # 50 node tiles

# Padded x table: row 0 = zero row (low window), row N+1 = zero row
# (high window), padded to a multiple of NCORES for the AllGather shard.
NROWS_PAD = 50304
XSHARD = NROWS_PAD // NCORES  # 6288
LO_SIZE = 32768
HI_OFF = NROWS_PAD - LO_SIZE  # 17536
ZHI = N + 1  # index of the high-window zero row (50001)

# Weight blob rows: w1(259) + w2(128) + w3(259) + w4(128) = 774, pad 776
WROWS = 776
WSHARD = WROWS // NCORES  # 97
W1_OFF, W2_OFF, W3_OFF, W4_OFF = 0, 259, 387, 646

BF16 = mybir.dt.bfloat16
F32 = mybir.dt.float32
I16 = mybir.dt.int16
NPBF16 = ml_dtypes.bfloat16

_cache = {}


def _blob_layout(T_B):
    """Byte-identical input blob layout shared by host packing and the
    device kernel.  All sections are 2-byte dtypes (bf16 / int16); offsets
    are in 2-byte elements, 256-element (512 B) aligned."""
    win_cap = T_B * P
    E_pad = NWIN * win_cap
    C = win_cap // 16
    CN = NODE_SLOTS // 16
    sections = [
        ("xshard", XSHARD, D),
        ("wshard", WSHARD, KO),
        ("gidx", NWIN * 16, 2 * C),
        ("slotsW", NWIN * P, T_B),
        ("battrW", NWIN * P, T_B * A),
        ("amfW", AM, E_pad),
        ("nidx", 16, 2 * CN),
        ("nanfT", AM, NODE_SLOTS),
        ("nattrW", P, NNT * A),
        ("bias", 4, D),
    ]
    layout = {}
    off = 0
    for name, r, c in sections:
        layout[name] = (off, r, c)
        off += -(r * c) // -256 * 256  # round up to 256 elements
    return layout, off


# --------------------------------------------------------------------------
# Host-side preparation
# --------------------------------------------------------------------------

def _assign_nodes(dst):
    """Snake round-robin of nodes (sorted by edge count desc) over the
    NCORES*NWIN bins: balances per-bin edge counts, gives every bin
    exactly ceil(N/nbins) <= SLOTS nodes, fully vectorized."""
    counts = np.bincount(dst, minlength=N)
    order = np.argsort(-counts, kind="stable")
    nbins = NCORES * NWIN
    k = np.arange(N)
    rnd, pos = k // nbins, k % nbins
    bins = np.where(rnd % 2 == 0, pos, nbins - 1 - pos).astype(np.int32)
    node2bin = np.empty(N, dtype=np.int32)
    node2slot = np.empty(N, dtype=np.int32)
    node2bin[order] = bins
    node2slot[order] = rnd.astype(np.int32)
    return node2bin, node2slot


def _wrap16(a):
    """[NWIN, cap] -> [NWIN, 16, cap//16] int16: index i of each window
    wrapped to (partition i%16, column i//16).  The device replicates the
    16-partition block to all 8 gpsimd groups."""
    nwin, cap = a.shape
    return np.ascontiguousarray(
        a.reshape(nwin, cap // 16, 16).transpose(0, 2, 1).astype(np.int16))


def _split_idx(id1):
    """Padded-table row ids -> (lo, hi) int16 gather indices.  Whichever
    window does not contain the row points at that window's zero row."""
    lo = np.where(id1 < LO_SIZE, id1, 0).astype(np.int16)
    hi = np.where(id1 >= LO_SIZE, id1 - HI_OFF, ZHI - HI_OFF).astype(np.int16)
    return lo, hi


def _prepare(x, edge_attr, node_attr, amf, anf, W1, b1, W2, b2, W3, b3, W4, b4,
             edge_index):
    x = np.asarray(x, dtype=np.float32)
    edge_attr = np.asarray(edge_attr, dtype=np.float32)
    node_attr = np.asarray(node_attr, dtype=np.float32)
    amf = np.asarray(amf, dtype=np.float32)
    anf = np.asarray(anf, dtype=np.float32)
    src = np.asarray(edge_index[0], dtype=np.int64).astype(np.int32)
    dst = np.asarray(edge_index[1], dtype=np.int64).astype(np.int32)

    node2bin, node2slot = _assign_nodes(dst)
    node_core = node2bin // NWIN
    node_gslot = (node2bin % NWIN) * SLOTS + node2slot

    e_bin = node2bin[dst]
    e_order = np.argsort(e_bin, kind="stable")
    e_bin_sorted = e_bin[e_order]
    bin_cnt = np.bincount(e_bin_sorted, minlength=NCORES * NWIN)
    T_B = int(np.ceil(bin_cnt.max() / P))
    win_cap = T_B * P
    E_pad = NWIN * win_cap

    bin_starts = np.zeros(NCORES * NWIN + 1, dtype=np.int64)
    np.cumsum(bin_cnt, out=bin_starts[1:])
    offs_in_bin = np.arange(len(e_order)) - bin_starts[e_bin_sorted]
    pos = (e_bin_sorted % NWIN) * win_cap + offs_in_bin
    core_of_edge = e_bin_sorted // NWIN
    gpos = core_of_edge * E_pad + pos

    src_s, dst_s = src[e_order], dst[e_order]

    # per-(core,slot) padded edge arrays; 0 / -1 sentinels for padding
    id1s = np.zeros(NCORES * E_pad, np.int32)
    slots_a = np.full(NCORES * E_pad, -1.0, np.float32)
    battr_a = np.zeros((NCORES * E_pad, A), np.float32)
    amf_a = np.zeros((NCORES * E_pad, AM), np.float32)
    id1s[gpos] = src_s + 1
    slots_a[gpos] = node2slot[dst_s]
    battr_a[gpos] = edge_attr[e_order]
    amf_a[gpos] = amf[e_order]

    id1s = id1s.reshape(NCORES, NWIN, win_cap)
    C = win_cap // 16

    # padded x table (shared across cores; shipped sharded)
    xstage = np.zeros((NROWS_PAD, D), NPBF16)
    xstage[1:N + 1] = x.astype(NPBF16)

    # weight blob
    wblob = np.zeros((WROWS, KO), NPBF16)
    wblob[W1_OFF:W1_OFF + MIN_DIM] = np.asarray(W1, np.float32).reshape(MIN_DIM, KO).astype(NPBF16)
    wblob[W2_OFF:W2_OFF + D] = np.asarray(W2, np.float32).reshape(D, KO).astype(NPBF16)
    wblob[W3_OFF:W3_OFF + UIN_DIM] = np.asarray(W3, np.float32).reshape(UIN_DIM, KO).astype(NPBF16)
    wblob[W4_OFF:W4_OFF + D] = np.asarray(W4, np.float32).reshape(D, KO).astype(NPBF16)

    bias4 = np.stack([np.asarray(b, np.float32) for b in (b1, b2, b3, b4)]
                     ).astype(NPBF16)  # [4, D]

    layout, blob_elems = _blob_layout(T_B)

    def pack(parts):
        blob = np.zeros(blob_elems, np.int16)
        for name, arr in parts.items():
            off, r, c = layout[name]
            blob[off:off + r * c] = arr.view(np.int16).ravel()
        return blob

    in_maps = []
    slot2node = np.full((NCORES, NODE_SLOTS), -1, dtype=np.int64)
    for c in range(NCORES):
        slo, shi = _split_idx(id1s[c])
        gidx = np.concatenate(
            [_wrap16(a) for a in (slo, shi)], axis=2)  # [NWIN,16,2C]
        gidx = np.ascontiguousarray(gidx.reshape(NWIN * 16, 2 * C))

        slotsW = slots_a.reshape(NCORES, NWIN, T_B, P)[c].transpose(0, 2, 1)
        slotsW = np.ascontiguousarray(slotsW.reshape(NWIN * P, T_B).astype(NPBF16))
        battrW = battr_a.reshape(NCORES, NWIN, T_B, P, A)[c].transpose(0, 2, 1, 3)
        battrW = np.ascontiguousarray(battrW.reshape(NWIN * P, T_B * A).astype(NPBF16))
        amfW = np.ascontiguousarray(
            amf_a.reshape(NCORES, E_pad, AM)[c].T.astype(NPBF16))

        # node side
        nodes_c = np.nonzero(node_core == c)[0]
        gs = node_gslot[nodes_c]
        slot2node[c, gs] = nodes_c
        nid1 = np.zeros(NODE_SLOTS, np.int32)
        nid1[gs] = nodes_c + 1
        nlo, nhi = _split_idx(nid1[None, :])
        nidx = np.ascontiguousarray(np.concatenate(
            [_wrap16(nlo)[0], _wrap16(nhi)[0]], axis=1))  # [16, 2*CN]
        nanfT = np.zeros((AM, NODE_SLOTS), NPBF16)
        nanfT[:, gs] = anf[nodes_c].T.astype(NPBF16)
        nattr_a = np.zeros((NODE_SLOTS, A), np.float32)
        nattr_a[gs] = node_attr[nodes_c]
        nattrW = np.ascontiguousarray(
            nattr_a.reshape(NNT, P, A).transpose(1, 0, 2).reshape(P, NNT * A).astype(NPBF16))

        in_maps.append({"blob": pack({
            "xshard": np.ascontiguousarray(xstage[c * XSHARD:(c + 1) * XSHARD]),
            "wshard": np.ascontiguousarray(wblob[c * WSHARD:(c + 1) * WSHARD]),
            "gidx": gidx,
            "slotsW": slotsW,
            "battrW": battrW,
            "amfW": amfW,
            "nidx": nidx,
            "nanfT": np.ascontiguousarray(nanfT),
            "nattrW": nattrW,
            "bias": bias4,
        })})
    return in_maps, slot2node, T_B


# --------------------------------------------------------------------------
# Device kernel builder
# --------------------------------------------------------------------------

def _build(T_B):
    win_cap = T_B * P
    E_pad = NWIN * win_cap
    C = win_cap // 16
    CN = NODE_SLOTS // 16

    nc = bacc.Bacc("TRN2", target_bir_lowering=False, debug=False,
                   num_devices=NCORES)

    layout, blob_elems = _blob_layout(T_B)
    d_blob = nc.dram_tensor("blob", [blob_elems], I16, kind="ExternalInput")

    def sect(name, dtype):
        off, r, c = layout[name]
        ap = bass.AP(d_blob.ap().tensor, off, [[c, r], [1, c]])
        return ap if dtype == I16 else ap.bitcast(dtype)

    d_xshard = sect("xshard", BF16)
    d_wshard = sect("wshard", BF16)
    d_gidx = sect("gidx", I16)
    d_slotsW = sect("slotsW", BF16)
    d_battrW = sect("battrW", BF16)
    d_amfW = sect("amfW", BF16)
    d_nidx = sect("nidx", I16)
    d_nanfT = sect("nanfT", BF16)
    d_nattrW = sect("nattrW", BF16)
    d_bias = sect("bias", BF16)
    d_out = nc.dram_tensor("out", [NODE_SLOTS, D], BF16, kind="ExternalOutput")

    d_xsh_i = nc.dram_tensor("xsh_i", [XSHARD, D], BF16)
    d_xfull = nc.dram_tensor("xfull", [NROWS_PAD, D], BF16, addr_space="Shared")
    d_wsh_i = nc.dram_tensor("wsh_i", [WSHARD, KO], BF16)
    d_wfull = nc.dram_tensor("wfull", [WROWS, KO], BF16, addr_space="Shared")

    mult = mybir.AluOpType.mult
    add = mybir.AluOpType.add
    iseq = mybir.AluOpType.is_equal
    silu = mybir.ActivationFunctionType.Silu
    groups = [list(range(NCORES))]

    with tile.TileContext(nc) as tc:
        with (
            tc.tile_pool(name="const", bufs=1) as cpool,
            tc.tile_pool(name="ain", bufs=2) as apool,
            tc.tile_pool(name="gth", bufs=2) as gpool,
            tc.tile_pool(name="work", bufs=3) as wpool,
            tc.tile_pool(name="cps", bufs=2, space="PSUM") as cps,
            tc.tile_pool(name="trps", bufs=2, space="PSUM") as trps,
            tc.tile_pool(name="aggps", bufs=1, space="PSUM") as aggps,
        ):
            # ---- replicate x and weights across cores ----
            nc.sync.dma_start(d_xsh_i.ap(), d_xshard)
            nc.gpsimd.collective_compute(
                "AllGather", mybir.AluOpType.bypass, groups,
                ins=[d_xsh_i.ap()], outs=[d_xfull.ap()])
            nc.sync.dma_start(d_wsh_i.ap(), d_wshard)
            nc.gpsimd.collective_compute(
                "AllGather", mybir.AluOpType.bypass, groups,
                ins=[d_wsh_i.ap()], outs=[d_wfull.ap()])

            x_lo = d_xfull.ap()[0:LO_SIZE, :]
            x_hi = d_xfull.ap()[HI_OFF:NROWS_PAD, :]

            # ---- constants resident in SBUF ----
            ident = cpool.tile([P, P], BF16, tag="ident", name="ident")
            make_identity(nc, ident[:])

            def wtile(rows, off, tag):
                t = cpool.tile([rows, KO], BF16, tag=tag, name=tag)
                nc.sync.dma_start(t[:], d_wfull.ap()[off:off + rows, :])
                return t

            w1c = [wtile(P, W1_OFF, "w1c0"), wtile(P, W1_OFF + P, "w1c1"),
                   wtile(AM, W1_OFF + 2 * P, "w1c2")]
            w2c = [wtile(P, W2_OFF, "w2c")]
            w3c = [wtile(P, W3_OFF, "w3c0"), wtile(P, W3_OFF + P, "w3c1"),
                   wtile(AM, W3_OFF + 2 * P, "w3c2")]
            w4c = [wtile(P, W4_OFF, "w4c")]

            # biases: [4, D] bf16 rows -> [P, D] f32 via K=1 ones matmul
            ones1 = cpool.tile([1, P], BF16, tag="ones1", name="ones1")
            nc.vector.memset(ones1[:], 1.0)
            btile = [cpool.tile([P, D], F32, tag=f"b{i}r", name=f"b{i}r")
                     for i in range(4)]
            for i in range(4):
                brow = cpool.tile([1, D], BF16, tag=f"brow{i}", name=f"brow{i}")
                nc.sync.dma_start(brow[:], d_bias[i:i + 1, :])
                bps = aggps.tile([P, D], F32, tag="bps", name="bps")
                nc.tensor.matmul(bps[:], lhsT=ones1[:], rhs=brow[:],
                                 start=True, stop=True)
                nc.vector.tensor_copy(btile[i][:], bps[:])

            cols_i = cpool.tile([P, SLOTS], mybir.dt.int32, tag="colsi", name="colsi")
            nc.gpsimd.iota(cols_i[:], pattern=[[1, SLOTS]], base=0,
                           channel_multiplier=0)
            cols_f = cpool.tile([P, SLOTS], F32, tag="colsf", name="colsf")
            nc.vector.tensor_copy(cols_f[:], cols_i[:])

            # per-partition row index columns (for the transposed one-hot ST)
            prow_i = cpool.tile([P, 2], mybir.dt.int32, tag="prowi", name="prowi")
            nc.gpsimd.iota(prow_i[:], pattern=[[128, 2]], base=0,
                           channel_multiplier=1)
            prow_f = cpool.tile([P, 2], F32, tag="prowf", name="prowf")
            nc.vector.tensor_copy(prow_f[:], prow_i[:])

            aggT = cpool.tile([P, NODE_SLOTS], BF16, tag="aggT", name="aggT")

            # ---- helper: one TP layer tile ----
            def tp_layer(chunks, wchunks, bt, bias_rep, out_tile, do_silu):
                cpsum = cps.tile([P, KO], F32, tag="c", name="c")
                nch = len(chunks)
                for ci in range(nch):
                    for h in range(2):
                        nc.tensor.matmul(
                            cpsum[:, h * 512:(h + 1) * 512],
                            lhsT=chunks[ci],
                            rhs=wchunks[ci][:, h * 512:(h + 1) * 512],
                            start=(ci == 0),
                            stop=(ci == nch - 1),
                        )
                acc = wpool.tile([P, D], F32, tag="acc", name="acc")
                nc.vector.scalar_tensor_tensor(
                    acc[:], cpsum[:, 0:D], bt[:, 0:1], bias_rep[:], mult, add)
                for k in range(1, A):
                    nc.vector.scalar_tensor_tensor(
                        acc[:], cpsum[:, k * D:(k + 1) * D], bt[:, k:k + 1],
                        acc[:], mult, add)
                if do_silu:
                    nc.scalar.activation(out_tile[:], acc[:], silu)
                else:
                    nc.vector.tensor_copy(out_tile[:], acc[:])

            def transpose_to(src_bf16):
                tps = trps.tile([P, P], BF16, tag="tr", name="tr")
                nc.tensor.transpose(tps[:], src_bf16[:], ident[:])
                dst = wpool.tile([P, P], BF16, tag="mT", name="mT")
                nc.vector.tensor_copy(dst[:], tps[:])
                return dst

            def rep16(dst_tile, dram_ap):
                """Load a [16, X] int16 DRAM block into all 8 16-partition
                groups of dst_tile (dma_gather reads indices per-group)."""
                for k in range(8):
                    nc.sync.dma_start(dst_tile[16 * k:16 * (k + 1), :], dram_ap)

            def gather_pair(dst_tile, lo_cols, hi_cols, idxt, n_idx, tag):
                """dst = xfull[ids] via dual-window dma_gather + add."""
                tmp = gpool.tile(list(dst_tile.shape), BF16, tag=tag, name=tag)
                nc.gpsimd.dma_gather(
                    dst_tile[:].unsqueeze(1), x_lo, idxt[:, lo_cols[0]:lo_cols[1]],
                    num_idxs=n_idx, num_idxs_reg=n_idx, elem_size=D,
                    transpose=True, single_packet=False)
                nc.gpsimd.dma_gather(
                    tmp[:].unsqueeze(1), x_hi, idxt[:, hi_cols[0]:hi_cols[1]],
                    num_idxs=n_idx, num_idxs_reg=n_idx, elem_size=D,
                    transpose=True, single_packet=False)
                nc.vector.tensor_tensor(dst_tile[:], dst_tile[:], tmp[:], add)

            # ---- owned-node features (needed by both phases) ----
            nidxt = cpool.tile([P, 2 * CN], I16, tag="nidxt", name="nidxt")
            rep16(nidxt, d_nidx)
            nxT = cpool.tile([P, NODE_SLOTS], BF16, tag="nxT", name="nxT")
            gather_pair(nxT, (0, CN), (CN, 2 * CN), nidxt, NODE_SLOTS, "gnx")

            # ---- edge phase ----
            for w in range(NWIN):
                idxt = apool.tile([P, 2 * C], I16, tag="idxt", name="idxt")
                rep16(idxt, d_gidx[w * 16:(w + 1) * 16, :])
                slt_b = apool.tile([P, T_B], BF16, tag="sltb", name="sltb")
                nc.sync.dma_start(slt_b[:], d_slotsW[w * P:(w + 1) * P, :])
                slt = wpool.tile([P, T_B], F32, tag="sltf", name="sltf")
                nc.vector.tensor_copy(slt[:], slt_b[:])
                bat_b = apool.tile([P, T_B * A], BF16, tag="batb", name="batb")
                nc.sync.dma_start(bat_b[:], d_battrW[w * P:(w + 1) * P, :])
                bat = wpool.tile([P, T_B * A], F32, tag="batf", name="batf")
                nc.vector.tensor_copy(bat[:], bat_b[:])
                amt = apool.tile([AM, win_cap], BF16, tag="amt", name="amt")
                nc.sync.dma_start(
                    amt[:], d_amfW[:, w * win_cap:(w + 1) * win_cap])

                # Ya[s, ko] = sum_d xown[d, w*SLOTS+s] * W1a[d, ko]; the
                # per-edge x_i contribution is then ST-selected by slot.
                ya = []
                for h in range(2):
                    ya_ps = cps.tile([P, KO], F32, tag="c", name="c")
                    for q in range(2):
                        nc.tensor.matmul(
                            ya_ps[:, q * 512:(q + 1) * 512],
                            lhsT=nxT[:, w * SLOTS + h * P:w * SLOTS + (h + 1) * P],
                            rhs=w1c[0][:, q * 512:(q + 1) * 512],
                            start=True, stop=True)
                    ya_sb = apool.tile([P, KO], BF16, tag=f"ya{h}", name=f"ya{h}")
                    nc.vector.tensor_copy(ya_sb[:], ya_ps[:])
                    ya.append(ya_sb)

                xj = gpool.tile([P, win_cap], BF16, tag="xj", name="xj")
                gather_pair(xj, (0, C), (C, 2 * C), idxt, win_cap, "ghj")

                agg_ps = aggps.tile([P, SLOTS], F32, tag="agg", name="agg")
                for j in range(T_B):
                    # transposed one-hot: ST[s, e] = (slot[e] == prow[s])
                    tt_ps = trps.tile([P, P], BF16, tag="tr", name="tr")
                    nc.tensor.transpose(
                        tt_ps[:], slt_b[:, j:j + 1].to_broadcast([P, P]),
                        ident[:])
                    ttf = wpool.tile([P, P], F32, tag="ttf", name="ttf")
                    nc.vector.tensor_copy(ttf[:], tt_ps[:])
                    st_lo = wpool.tile([P, P], BF16, tag="stlo", name="stlo")
                    nc.vector.tensor_scalar(
                        st_lo[:], ttf[:], prow_f[:, 0:1], None, iseq)
                    st_hi = wpool.tile([P, P], BF16, tag="sthi", name="sthi")
                    nc.vector.tensor_scalar(
                        st_hi[:], ttf[:], prow_f[:, 1:2], None, iseq)

                    m1 = wpool.tile([P, D], BF16, tag="m1", name="m1")
                    tp_layer([st_lo, st_hi,
                              xj[:, j * P:(j + 1) * P],
                              amt[:, j * P:(j + 1) * P]],
                             [ya[0], ya[1], w1c[1], w1c[2]],
                             bat[:, j * A:(j + 1) * A], btile[0], m1, True)
                    m1T = transpose_to(m1)
                    m2 = wpool.tile([P, D], BF16, tag="m2", name="m2")
                    tp_layer([m1T], w2c, bat[:, j * A:(j + 1) * A], btile[1],
                             m2, True)

                    St = wpool.tile([P, SLOTS], BF16, tag="St", name="St")
                    nc.vector.tensor_scalar(
                        St[:], cols_f[:], slt[:, j:j + 1], None, iseq)
                    nc.tensor.matmul(
                        agg_ps[:], lhsT=m2[:], rhs=St[:],
                        start=(j == 0), stop=(j == T_B - 1))
                nc.vector.tensor_copy(
                    aggT[:, w * SLOTS:(w + 1) * SLOTS], agg_ps[:])

            # ---- node phase ----
            nanf = cpool.tile([AM, NODE_SLOTS], BF16, tag="nanf", name="nanf")
            nc.sync.dma_start(nanf[:], d_nanfT)
            nat_b = cpool.tile([P, NNT * A], BF16, tag="natb", name="natb")
            nc.sync.dma_start(nat_b[:], d_nattrW)
            nat = cpool.tile([P, NNT * A], F32, tag="natf", name="natf")
            nc.vector.tensor_copy(nat[:], nat_b[:])

            for t in range(NNT):
                u = wpool.tile([P, D], BF16, tag="m1", name="m1")
                tp_layer([nxT[:, t * P:(t + 1) * P],
                          aggT[:, t * P:(t + 1) * P],
                          nanf[:, t * P:(t + 1) * P]],
                         w3c, nat[:, t * A:(t + 1) * A], btile[2], u, True)
                uT = transpose_to(u)
                out_t = wpool.tile([P, D], BF16, tag="outt", name="outt")
                tp_layer([uT], w4c, nat[:, t * A:(t + 1) * A], btile[3],
                         out_t, False)
                nc.sync.dma_start(d_out.ap()[t * P:(t + 1) * P, :], out_t[:])

    nc.compile()
    return nc


# --------------------------------------------------------------------------
# Cached PJRT runner
#
# bass_utils.run_bass_kernel_spmd re-wraps the NEFF in a fresh jax.jit on
# every call, so the terminal re-loads the (multi-MB) NEFF each time.
# Building the jit executable once per compiled kernel keeps the NEFF
# loaded; repeated calls then only pay input transfer + execution.
# --------------------------------------------------------------------------

def _make_runner(nc):
    """jit-once PJRT runner.  Unlike run_bass_via_pjrt it (a) reuses one
    loaded executable across calls (no per-call NEFF reload over the
    link) and (b) skips the donated zero output buffers — this kernel
    writes every output element, so uninitialized result buffers are
    fine and the zero upload is dead weight."""
    from concourse import bass2jax
    bass2jax.install_neuronx_cc_hook()
    assert not getattr(nc, "dbg_callbacks", None)
    partition_name = nc.partition_id_tensor.name if nc.partition_id_tensor else None

    in_names, out_names, out_avals = [], [], []
    for alloc in nc.m.functions[0].allocations:
        if not isinstance(alloc, mybir.MemoryLocationSet):
            continue
        name = alloc.memorylocations[0].name
        if alloc.kind == "ExternalInput":
            if name != partition_name:
                in_names.append(name)
        elif alloc.kind == "ExternalOutput":
            out_names.append(name)
            shape = tuple(alloc.tensor_shape)
            dtype = mybir.dt.np(alloc.dtype)
            out_avals.append(jax.core.ShapedArray(shape, dtype))
    all_in_names = in_names + ([partition_name] if partition_name else [])

    def _body(*args):
        operands = list(args)
        if partition_name is not None:
            operands.append(bass2jax.partition_id_tensor())
        outs = bass2jax._bass_exec_p.bind(
            *operands, out_avals=tuple(out_avals), in_names=tuple(all_in_names),
            out_names=tuple(out_names), lowering_input_output_aliases=(),
            sim_require_finite=True, sim_require_nnan=True, nc=nc)
        return tuple(outs)

    devices = jax.devices()[:NCORES]
    mesh = bass2jax.Mesh(np.asarray(devices), ("core",))
    in_specs = (bass2jax.PartitionSpec("core"),) * len(in_names)
    out_specs = (bass2jax.PartitionSpec("core"),) * len(out_names)
    sharded = jax.jit(
        bass2jax.shard_map(_body, mesh=mesh, in_specs=in_specs,
                           out_specs=out_specs, check_rep=False),
        keep_unused=True)

    def run(concat_in):
        out_arrs = sharded(*concat_in)
        return [
            {name: np.asarray(out_arrs[i]).reshape(NCORES, *out_avals[i].shape)[c]
             for i, name in enumerate(out_names)}
            for c in range(NCORES)
        ]

    run.in_names = in_names
    return run


def _concat_inputs(run, in_maps):
    return [np.concatenate([np.asarray(in_maps[c][name])
                            for c in range(NCORES)], axis=0)
            for name in run.in_names]


# --------------------------------------------------------------------------
# Entry point
# --------------------------------------------------------------------------

def kernel(x, edge_attr, node_attr, additional_message_features,
           additional_node_features, W1, b1, W2, b2, W3, b3, W4, b4,
           edge_index, batch=None):
    in_maps, slot2node, T_B = _prepare(
        x, edge_attr, node_attr, additional_message_features,
        additional_node_features, W1, b1, W2, b2, W3, b3, W4, b4, edge_index)

    if T_B not in _cache:
        nc = _build(T_B)
        _cache[T_B] = (nc, _make_runner(nc))
    nc, run = _cache[T_B]

    concat_in = _concat_inputs(run, in_maps)
    results = run(concat_in)
    kernel.last = (run, concat_in, results)

    out = np.zeros((N, D), dtype=np.float32)
    for c in range(NCORES):
        oc = np.asarray(results[c]["out"], dtype=np.float32)
        mask = slot2node[c] >= 0
        out[slot2node[c][mask]] = oc[mask]
    return out
